# revision 9
# baseline (speedup 1.0000x reference)
"""Trainium2 Bass kernel for nn_Attention_62362925138174.

Reference computation (per batch b, with xf = x[b].reshape(C, N), N = H*W):
    q = Wq @ xf            [8,  N]
    k = Wk @ xf            [8,  N]
    v = Wv @ xf            [C,  N]
    score[n, m] = q[:, n] . k[:, m]
    P = softmax(score, axis=n)          (per-column softmax)
    out[c, m] = sum_n v[c, n] P[n, m]
    att = gamma * out + xf

Kernel strategy (8 cores = 4 batches x 2 column-halves of N):
  - Score via the rank-8 q^T k form with 4x PE row tiling: q and k are
    projected on device into partition groups {0,32,64,96} (one matmul with
    zero-padded replicated weights), so each 128-row score tile issues four
    concurrent 512-column matmuls on distinct 32-row PE groups.
  - exp() is split between ScalarE (exact activation) and VectorE using a
    Schraudolph fast-exp: bits16 = trunc(A*s + B) written as int16 is the
    bf16 bit pattern of ~exp(s) (+-3%, zero-mean after calibrating B; the
    softmax normalization cancels the scale, final rel err ~4e-5).
  - V@E accumulates with a ones-row appended to V^T so one PSUM chain gives
    both gamma*(V @ E) (gamma folded into Wv on the host) and colsum(E).
  - 1/colsum via exp(-ln(colsum)) on ScalarE; broadcast on GpSimd; residual
    add uses the exact f32 input.
"""

import numpy as np

import concourse.bass as bass
import concourse.bacc as bacc
import concourse.tile as tile
from concourse import mybir
from concourse.bass_utils import run_bass_kernel_spmd

# Problem shape (hardcoded per contract).
B, C, H, W = 4, 64, 64, 64
N = H * W           # 4096
MHALF = N // 2      # 2048 columns of the score/output handled per core
NT = N // 128       # 32 row-tiles of the score matrix
N_CORES = 8

F32 = mybir.dt.float32
BF16 = mybir.dt.bfloat16
I16 = mybir.dt.int16
_NP_BF16 = mybir.dt.np(BF16)

# Schraudolph fast-exp constants for bf16 bit patterns (DVE truncates on
# f32->int16 convert; B calibrated offline on the actual score distribution).
FEXP_A = 184.6650390625   # 2^7 / ln 2
FEXP_B = 16249.5

_PROGRAM = None


def _scalar_owns_exp(t: int, h: int) -> bool:
    """Split the 64 (t, h) exp tiles ~56/44 between ScalarE and VectorE."""
    return (2 * t + h) % 16 < 9


def _build_program() -> bass.Bass:
    nc = bacc.Bacc()

    xfp_d = nc.declare_dram_parameter("xfp", [C, N], BF16, isOutput=False)
    xkp_d = nc.declare_dram_parameter("xkp", [C, MHALF], BF16, isOutput=False)
    xkf_d = nc.declare_dram_parameter("xkf", [C, MHALF], F32, isOutput=False)
    wq4_d = nc.declare_dram_parameter("wq4", [C, 128], BF16, isOutput=False)
    wk4_d = nc.declare_dram_parameter("wk4", [C, 128], BF16, isOutput=False)
    wv_d = nc.declare_dram_parameter("wv", [C, C], BF16, isOutput=False)
    out_d = nc.declare_dram_parameter("out", [C, MHALF], F32, isOutput=True)

    EXP = mybir.ActivationFunctionType.Exp
    LN = mybir.ActivationFunctionType.Ln
    MULT = mybir.AluOpType.mult
    ADD = mybir.AluOpType.add

    from concourse.hw_specs import get_activation_tables

    act_sets = list(get_activation_tables(nc.m.arch))
    nle_id = act_sets.index("natural_log_exp_and_others")

    with TileCtx(nc) as (tc, sing, epool, apool, psS, psO):
        nc.scalar.add_instruction(
            mybir.InstLoadActFuncSet(
                name=nc.get_next_instruction_name(),
                act_func_set_id=nle_id,
                ins=[],
                outs=[],
            )
        )
        # ---- input loads; xfp/weights replicated into both 64-partition
        # halves (two DMAs from the same DRAM region), queues spread ----
        wq4_sb = sing.tile([128, 128], BF16, name="wq4_sb")
        wk4_sb = sing.tile([128, 128], BF16, name="wk4_sb")
        wv_sb = sing.tile([128, C], BF16, name="wv_sb")
        for g in range(2):
            nc.sync.dma_start(out=wq4_sb[64 * g : 64 * g + 64, :], in_=wq4_d[:, :])
            nc.sync.dma_start(out=wk4_sb[64 * g : 64 * g + 64, :], in_=wk4_d[:, :])
            nc.sync.dma_start(out=wv_sb[64 * g : 64 * g + 64, :], in_=wv_d[:, :])
        xfp_sb = sing.tile([128, N], BF16, name="xfp_sb")
        for i in range(4):
            cs = slice(i * 1024, (i + 1) * 1024)
            nc.scalar.dma_start(out=xfp_sb[0:64, cs], in_=xfp_d[:, cs])
            nc.sync.dma_start(out=xfp_sb[64:128, cs], in_=xfp_d[:, cs])
        xkp_sb = sing.tile([128, MHALF], BF16, name="xkp_sb")
        for g in range(2):
            nc.sync.dma_start(out=xkp_sb[64 * g : 64 * g + 64, :], in_=xkp_d[:, :])
        xkf_sb = sing.tile([C, MHALF], F32, name="xkf_sb")
        for i in range(2):
            cs = slice(i * 1024, (i + 1) * 1024)
            nc.sync.dma_start(out=xkf_sb[:, cs], in_=xkf_d[:, cs])

        # ---- k4 = Wk-projection of this core's half, replicated in the
        # partition groups by the zero-padded wk4 weight layout ----
        k4_sb = sing.tile([128, MHALF], BF16, name="k4_sb")
        for i in range(2):
            kp = psS.tile([128, 1024], F32, tag="S", name="kp")
            for cc in range(2):
                g = cc
                lo = i * 1024 + cc * 512
                nc.tensor.matmul(
                    kp[:, cc * 512 : (cc + 1) * 512],
                    lhsT=wk4_sb[64 * g : 64 * g + 64, :],
                    rhs=xkp_sb[64 * g : 64 * g + 64, lo : lo + 512],
                    start=True,
                    stop=True,
                    tile_position=(64 * g, 0),
                )
            nc.scalar.copy(out=k4_sb[:, i * 1024 : (i + 1) * 1024], in_=kp)

        # ---- q4 = Wq-projection of all N columns, same replica layout ----
        q4_sb = sing.tile([128, N], BF16, name="q4_sb")
        for qi in range(4):
            qp = psS.tile([128, 1024], F32, tag="S", name="qp")
            for cc in range(2):
                g = cc
                lo = qi * 1024 + cc * 512
                nc.tensor.matmul(
                    qp[:, cc * 512 : (cc + 1) * 512],
                    lhsT=wq4_sb[64 * g : 64 * g + 64, :],
                    rhs=xfp_sb[64 * g : 64 * g + 64, lo : lo + 512],
                    start=True,
                    stop=True,
                    tile_position=(64 * g, 0),
                )
            qsl = slice(qi * 1024, (qi + 1) * 1024)
            if qi < 2:
                nc.scalar.copy(out=q4_sb[:, qsl], in_=qp)
            else:
                nc.vector.tensor_copy(out=q4_sb[:, qsl], in_=qp)

        # ---- vaugT[n, 0:64] = (gamma*Wv @ xf)^T tile, vaugT[n, 64] = 1 ----
        vaug_sb = sing.tile([128, NT * 65], BF16, name="vaug_sb")
        vaug3 = vaug_sb.rearrange("p (t u) -> p t u", u=65)
        nc.vector.memset(vaug3[:, :, 64:65], 1.0)

        def emit_vt_chunk(vv):
            # one PE row-group per chunk: concurrent tile_position matmuls
            # into the same PSUM bank are fatal on HW, sequential are fine
            vtp = psS.tile([128, 512], F32, tag="S", name="vtp")
            g = vv % 2
            for i in range(8):
                t = vv * 8 + i
                nc.tensor.matmul(
                    vtp[:, i * 64 : (i + 1) * 64],
                    lhsT=xfp_sb[64 * g : 64 * g + 64, t * 128 : (t + 1) * 128],
                    rhs=wv_sb[64 * g : 64 * g + 64, :],
                    start=True,
                    stop=True,
                    tile_position=(64 * g, 0),
                )
            nc.vector.tensor_copy(
                out=vaug3[:, vv * 8 : (vv + 1) * 8, 0:64],
                in_=vtp.rearrange("p (i u) -> p i u", u=64),
            )

        emit_vt_chunk(0)

        # ---- main loop: score -> exp -> accumulate V_aug @ E ----
        O_ps = psO.tile([65, MHALF], F32, name="O_ps")
        for t in range(NT):
            Es = []
            for h in range(2):
                S = psS.tile([128, 1024], F32, tag="S", name="S_ps")
                for cc in range(2):
                    r = 2 * h + cc
                    nc.tensor.matmul(
                        S[:, cc * 512 : (cc + 1) * 512],
                        lhsT=q4_sb[32 * r : 32 * r + 8, t * 128 : (t + 1) * 128],
                        rhs=k4_sb[32 * r : 32 * r + 8, r * 512 : (r + 1) * 512],
                        start=True,
                        stop=True,
                        tile_position=(32 * r, 0),
                    )
                if _scalar_owns_exp(t, h):
                    E = epool.tile([128, 1024], BF16, tag="E", name="E_sb")
                    nc.scalar.activation(out=E, in_=S, func=EXP)
                    Es.append(E)
                else:
                    E = epool.tile([128, 1024], I16, tag="E", name="Ei_sb")
                    nc.vector.tensor_scalar(E, S, FEXP_A, FEXP_B, MULT, ADD)
                    Es.append(E.bitcast(BF16))
            if t in (1, 3, 5):
                emit_vt_chunk((t + 1) // 2)
            va_t = vaug3[:, t, :]
            for h in range(2):
                for cc in range(2):
                    r = 2 * h + cc
                    nc.tensor.matmul(
                        O_ps[:, r * 512 : (r + 1) * 512],
                        lhsT=va_t,
                        rhs=Es[h][:, cc * 512 : (cc + 1) * 512],
                        start=(t == 0),
                        stop=(t == NT - 1),
                    )

        # ---- normalize + residual, store (per m-half, pipelined) ----
        for half in range(2):
            hsl = slice(half * 1024, (half + 1) * 1024)
            lnt = apool.tile([1, 1024], F32, tag="lnt", name="lnt")
            nc.scalar.activation(out=lnt, in_=O_ps[64:65, hsl], func=LN)
            rcp = apool.tile([1, 1024], BF16, tag="rcp", name="rcp")
            nc.scalar.activation(out=rcp, in_=lnt, func=EXP, scale=-1.0)
            bcs = apool.tile([C, 1024], BF16, tag="bcs", name="bcs")
            nc.gpsimd.partition_broadcast(bcs, rcp)
            tmp = apool.tile([C, 1024], F32, tag="tmp", name="tmp")
            nc.vector.tensor_mul(tmp, O_ps[0:C, hsl], bcs)
            att = apool.tile([C, 1024], F32, tag="att", name="att")
            nc.vector.tensor_add(att, tmp, xkf_sb[:, hsl])
            nc.sync.dma_start(out=out_d[:, hsl], in_=att)

    nc.finalize()
    return nc


class TileCtx:
    """TileContext plus the tile pools used by the kernel."""

    def __init__(self, nc: bass.Bass):
        self.nc = nc

    def __enter__(self):
        from contextlib import ExitStack

        self._stack = ExitStack()
        tc = self._stack.enter_context(tile.TileContext(self.nc))
        sing = self._stack.enter_context(tc.tile_pool(name="sing", bufs=1))
        epool = self._stack.enter_context(tc.tile_pool(name="epool", bufs=6))
        apool = self._stack.enter_context(tc.tile_pool(name="apool", bufs=4))
        psS = self._stack.enter_context(tc.tile_pool(name="psS", bufs=2, space="PSUM"))
        psO = self._stack.enter_context(tc.tile_pool(name="psO", bufs=1, space="PSUM"))
        return tc, sing, epool, apool, psS, psO

    def __exit__(self, *exc):
        return self._stack.__exit__(*exc)


def get_program() -> bass.Bass:
    global _PROGRAM
    if _PROGRAM is None:
        _PROGRAM = _build_program()
    return _PROGRAM


def make_in_maps(x, Wq, Wk, Wv, gamma):
    """Shard the full inputs into per-core input maps (host-side prep only:
    reshape/slice, replicated zero-padded weight layouts, cast to bf16)."""
    x = np.ascontiguousarray(np.asarray(x, dtype=np.float32))
    Wq = np.asarray(Wq, dtype=np.float32)
    Wk = np.asarray(Wk, dtype=np.float32)
    Wv = np.asarray(Wv, dtype=np.float32)
    gamma = float(np.asarray(gamma, dtype=np.float32).reshape(()))

    def rep4(Wm):  # [8, 64] -> [64, 128] with W^T at free-cols 32a..32a+8
        out = np.zeros((C, 128), dtype=_NP_BF16)
        for a in range(4):
            out[:, 32 * a : 32 * a + 8] = Wm.T.astype(_NP_BF16)
        return out

    wq4 = rep4(Wq)
    wk4 = rep4(Wk)
    wv = np.ascontiguousarray((gamma * Wv.T).astype(_NP_BF16))  # [64, 64]

    in_maps = []
    for core in range(N_CORES):
        b, h = divmod(core, 2)
        xf = x[b].reshape(C, N)
        xk = xf[:, h * MHALF : (h + 1) * MHALF]
        in_maps.append(
            {
                "xfp": xf.astype(_NP_BF16),
                "xkp": np.ascontiguousarray(xk.astype(_NP_BF16)),
                "xkf": np.ascontiguousarray(xk),
                "wq4": wq4,
                "wk4": wk4,
                "wv": wv,
            }
        )
    return in_maps


def gather(results):
    out = np.empty((B, C, N), dtype=np.float32)
    for core in range(N_CORES):
        b, h = divmod(core, 2)
        out[b][:, h * MHALF : (h + 1) * MHALF] = results[core]["out"]
    return out.reshape(B, C, H, W)


def run(inputs, **spmd_kwargs):
    nc = get_program()
    in_maps = make_in_maps(
        inputs["x"], inputs["Wq"], inputs["Wk"], inputs["Wv"], inputs["gamma"]
    )
    res = run_bass_kernel_spmd(nc, in_maps, core_ids=list(range(N_CORES)), **spmd_kwargs)
    return gather(res.results), res


def kernel(x, Wq, Wk, Wv, gamma):
    out, _ = run({"x": x, "Wq": Wq, "Wk": Wk, "Wv": Wv, "gamma": gamma})
    return out


# revision 11
# speedup vs baseline: 1.3661x; 1.3661x over previous
"""Trainium2 Bass kernel for nn_Attention_62362925138174.

Reference computation (per batch b, with xf = x[b].reshape(C, N), N = H*W):
    q = Wq @ xf            [8,  N]
    k = Wk @ xf            [8,  N]
    v = Wv @ xf            [C,  N]
    score[n, m] = q[:, n] . k[:, m]
    P = softmax(score, axis=n)          (per-column softmax)
    out[c, m] = sum_n v[c, n] P[n, m]
    att = gamma * out + xf

Kernel strategy (8 cores = 4 batches x 2 column-halves of N):
  - Score via the rank-8 q^T k form with 4x PE row tiling: q and k are
    projected on device into partition groups {0,32,64,96} (one matmul with
    zero-padded replicated weights), so each 128-row score tile issues four
    concurrent 512-column matmuls on distinct 32-row PE groups.
  - exp() is split between ScalarE (exact activation) and VectorE using a
    Schraudolph fast-exp: bits16 = trunc(A*s + B) written as int16 is the
    bf16 bit pattern of ~exp(s) (+-3%, zero-mean after calibrating B; the
    softmax normalization cancels the scale, final rel err ~4e-5).
  - V@E accumulates with a ones-row appended to V^T so one PSUM chain gives
    both gamma*(V @ E) (gamma folded into Wv on the host) and colsum(E).
  - 1/colsum via exp(-ln(colsum)) on ScalarE; broadcast on GpSimd; residual
    add uses the exact f32 input.
"""

import numpy as np

import concourse.bass as bass
import concourse.bacc as bacc
import concourse.tile as tile
from concourse import mybir
from concourse.bass_utils import run_bass_kernel_spmd

# Problem shape (hardcoded per contract).
B, C, H, W = 4, 64, 64, 64
N = H * W           # 4096
MHALF = N // 2      # 2048 columns of the score/output handled per core
NT = N // 128       # 32 row-tiles of the score matrix
N_CORES = 8

F32 = mybir.dt.float32
BF16 = mybir.dt.bfloat16
I16 = mybir.dt.int16
_NP_BF16 = mybir.dt.np(BF16)

# Schraudolph fast-exp constants for bf16 bit patterns (DVE truncates on
# f32->int16 convert; B calibrated offline on the actual score distribution).
FEXP_A = 184.6650390625   # 2^7 / ln 2
FEXP_B = 16249.5

_PROGRAM = None


def _scalar_owns_exp(t: int, h: int) -> bool:
    """Split the 64 (t, h) exp tiles ~56/44 between ScalarE and VectorE."""
    return (2 * t + h) % 16 < 9


def _build_program() -> bass.Bass:
    nc = bacc.Bacc()

    xfp_d = nc.declare_dram_parameter("xfp", [C, N], BF16, isOutput=False)
    xkp_d = nc.declare_dram_parameter("xkp", [C, MHALF], BF16, isOutput=False)
    xkf_d = nc.declare_dram_parameter("xkf", [C, MHALF], F32, isOutput=False)
    wq4_d = nc.declare_dram_parameter("wq4", [C, 128], BF16, isOutput=False)
    wk4_d = nc.declare_dram_parameter("wk4", [C, 128], BF16, isOutput=False)
    wv_d = nc.declare_dram_parameter("wv", [C, C], BF16, isOutput=False)
    out_d = nc.declare_dram_parameter("out", [C, MHALF], F32, isOutput=True)

    EXP = mybir.ActivationFunctionType.Exp
    LN = mybir.ActivationFunctionType.Ln
    MULT = mybir.AluOpType.mult
    ADD = mybir.AluOpType.add

    from concourse.hw_specs import get_activation_tables

    act_sets = list(get_activation_tables(nc.m.arch))
    nle_id = act_sets.index("natural_log_exp_and_others")

    with TileCtx(nc) as (tc, sing, epool, apool, psS, psO):
        nc.scalar.add_instruction(
            mybir.InstLoadActFuncSet(
                name=nc.get_next_instruction_name(),
                act_func_set_id=nle_id,
                ins=[],
                outs=[],
            )
        )
        # ---- input loads; xfp/weights replicated into both 64-partition
        # halves (two DMAs from the same DRAM region), queues spread ----
        wq4_sb = sing.tile([128, 128], BF16, name="wq4_sb")
        wk4_sb = sing.tile([128, 128], BF16, name="wk4_sb")
        wv_sb = sing.tile([128, C], BF16, name="wv_sb")
        for g in range(2):
            nc.sync.dma_start(out=wq4_sb[64 * g : 64 * g + 64, :], in_=wq4_d[:, :])
            nc.sync.dma_start(out=wk4_sb[64 * g : 64 * g + 64, :], in_=wk4_d[:, :])
            nc.sync.dma_start(out=wv_sb[64 * g : 64 * g + 64, :], in_=wv_d[:, :])
        xfp_sb = sing.tile([128, N], BF16, name="xfp_sb")
        for i in range(4):
            cs = slice(i * 1024, (i + 1) * 1024)
            nc.scalar.dma_start(out=xfp_sb[0:64, cs], in_=xfp_d[:, cs])
            nc.sync.dma_start(out=xfp_sb[64:128, cs], in_=xfp_d[:, cs])
        xkp_sb = sing.tile([128, MHALF], BF16, name="xkp_sb")
        for g in range(2):
            nc.sync.dma_start(out=xkp_sb[64 * g : 64 * g + 64, :], in_=xkp_d[:, :])
        xkf_sb = sing.tile([C, MHALF], F32, name="xkf_sb")
        for i in range(2):
            cs = slice(i * 1024, (i + 1) * 1024)
            nc.sync.dma_start(out=xkf_sb[:, cs], in_=xkf_d[:, cs])

        # ---- k4 = Wk-projection of this core's half, replicated in the
        # partition groups by the zero-padded wk4 weight layout ----
        k4_sb = sing.tile([128, MHALF], BF16, name="k4_sb")
        for i in range(2):
            kp = psS.tile([128, 1024], F32, tag="S", name="kp")
            for cc in range(2):
                g = cc
                lo = i * 1024 + cc * 512
                nc.tensor.matmul(
                    kp[:, cc * 512 : (cc + 1) * 512],
                    lhsT=wk4_sb[64 * g : 64 * g + 64, :],
                    rhs=xkp_sb[64 * g : 64 * g + 64, lo : lo + 512],
                    start=True,
                    stop=True,
                    tile_position=(64 * g, 0),
                )
            nc.scalar.copy(out=k4_sb[:, i * 1024 : (i + 1) * 1024], in_=kp)

        # ---- q4 = Wq-projection of all N columns, same replica layout ----
        q4_sb = sing.tile([128, N], BF16, name="q4_sb")
        for qi in range(4):
            qp = psS.tile([128, 1024], F32, tag="S", name="qp")
            for cc in range(2):
                g = cc
                lo = qi * 1024 + cc * 512
                nc.tensor.matmul(
                    qp[:, cc * 512 : (cc + 1) * 512],
                    lhsT=wq4_sb[64 * g : 64 * g + 64, :],
                    rhs=xfp_sb[64 * g : 64 * g + 64, lo : lo + 512],
                    start=True,
                    stop=True,
                    tile_position=(64 * g, 0),
                )
            qsl = slice(qi * 1024, (qi + 1) * 1024)
            if qi < 2:
                nc.scalar.copy(out=q4_sb[:, qsl], in_=qp)
            else:
                nc.vector.tensor_copy(out=q4_sb[:, qsl], in_=qp)

        # ---- vaugT[n, 0:64] = (gamma*Wv @ xf)^T tile, vaugT[n, 64] = 1 ----
        vaug_sb = sing.tile([128, NT * 65], BF16, name="vaug_sb")
        vaug3 = vaug_sb.rearrange("p (t u) -> p t u", u=65)
        nc.vector.memset(vaug3[:, :, 64:65], 1.0)

        def emit_vt_chunk(vv):
            # one PE row-group per chunk: concurrent tile_position matmuls
            # into the same PSUM bank are fatal on HW, sequential are fine
            vtp = psS.tile([128, 512], F32, tag="S", name="vtp")
            g = vv % 2
            for i in range(8):
                t = vv * 8 + i
                nc.tensor.matmul(
                    vtp[:, i * 64 : (i + 1) * 64],
                    lhsT=xfp_sb[64 * g : 64 * g + 64, t * 128 : (t + 1) * 128],
                    rhs=wv_sb[64 * g : 64 * g + 64, :],
                    start=True,
                    stop=True,
                    tile_position=(64 * g, 0),
                )
            nc.vector.tensor_copy(
                out=vaug3[:, vv * 8 : (vv + 1) * 8, 0:64],
                in_=vtp.rearrange("p (i u) -> p i u", u=64),
            )

        emit_vt_chunk(0)

        # ---- main loop: score -> exp -> accumulate V_aug @ E ----
        O_ps = psO.tile([65, MHALF], F32, name="O_ps")
        for t in range(NT):
            Es = []
            for h in range(2):
                S = psS.tile([128, 1024], F32, tag="S", name="S_ps")
                for cc in range(2):
                    r = 2 * h + cc
                    # full-array matmul: k4 is zero outside rows 0:8, so the
                    # 128-partition contraction picks out q-group 0 exactly
                    # (alternating PE tile configs costs ~400ns per switch)
                    nc.tensor.matmul(
                        S[:, cc * 512 : (cc + 1) * 512],
                        lhsT=q4_sb[:, t * 128 : (t + 1) * 128],
                        rhs=k4_sb[:, r * 512 : (r + 1) * 512],
                        start=True,
                        stop=True,
                    )
                if _scalar_owns_exp(t, h):
                    E = epool.tile([128, 1024], BF16, tag="E", name="E_sb")
                    nc.scalar.activation(out=E, in_=S, func=EXP)
                    Es.append(E)
                else:
                    E = epool.tile([128, 1024], I16, tag="E", name="Ei_sb")
                    nc.vector.tensor_scalar(E, S, FEXP_A, FEXP_B, MULT, ADD)
                    Es.append(E.bitcast(BF16))
            if t in (1, 3, 5):
                emit_vt_chunk((t + 1) // 2)
            va_t = vaug3[:, t, :]
            for h in range(2):
                for cc in range(2):
                    r = 2 * h + cc
                    nc.tensor.matmul(
                        O_ps[:, r * 512 : (r + 1) * 512],
                        lhsT=va_t,
                        rhs=Es[h][:, cc * 512 : (cc + 1) * 512],
                        start=(t == 0),
                        stop=(t == NT - 1),
                    )

        # ---- normalize + residual, store (per m-half, pipelined) ----
        for half in range(2):
            hsl = slice(half * 1024, (half + 1) * 1024)
            lnt = apool.tile([1, 1024], F32, tag="lnt", name="lnt")
            nc.scalar.activation(out=lnt, in_=O_ps[64:65, hsl], func=LN)
            rcp = apool.tile([1, 1024], BF16, tag="rcp", name="rcp")
            nc.scalar.activation(out=rcp, in_=lnt, func=EXP, scale=-1.0)
            bcs = apool.tile([C, 1024], BF16, tag="bcs", name="bcs")
            nc.gpsimd.partition_broadcast(bcs, rcp)
            tmp = apool.tile([C, 1024], F32, tag="tmp", name="tmp")
            nc.vector.tensor_mul(tmp, O_ps[0:C, hsl], bcs)
            att = apool.tile([C, 1024], F32, tag="att", name="att")
            nc.vector.tensor_add(att, tmp, xkf_sb[:, hsl])
            nc.sync.dma_start(out=out_d[:, hsl], in_=att)

    nc.finalize()
    return nc


class TileCtx:
    """TileContext plus the tile pools used by the kernel."""

    def __init__(self, nc: bass.Bass):
        self.nc = nc

    def __enter__(self):
        from contextlib import ExitStack

        self._stack = ExitStack()
        tc = self._stack.enter_context(tile.TileContext(self.nc))
        sing = self._stack.enter_context(tc.tile_pool(name="sing", bufs=1))
        epool = self._stack.enter_context(tc.tile_pool(name="epool", bufs=6))
        apool = self._stack.enter_context(tc.tile_pool(name="apool", bufs=4))
        psS = self._stack.enter_context(tc.tile_pool(name="psS", bufs=2, space="PSUM"))
        psO = self._stack.enter_context(tc.tile_pool(name="psO", bufs=1, space="PSUM"))
        return tc, sing, epool, apool, psS, psO

    def __exit__(self, *exc):
        return self._stack.__exit__(*exc)


def get_program() -> bass.Bass:
    global _PROGRAM
    if _PROGRAM is None:
        _PROGRAM = _build_program()
    return _PROGRAM


def make_in_maps(x, Wq, Wk, Wv, gamma):
    """Shard the full inputs into per-core input maps (host-side prep only:
    reshape/slice, replicated zero-padded weight layouts, cast to bf16)."""
    x = np.ascontiguousarray(np.asarray(x, dtype=np.float32))
    Wq = np.asarray(Wq, dtype=np.float32)
    Wk = np.asarray(Wk, dtype=np.float32)
    Wv = np.asarray(Wv, dtype=np.float32)
    gamma = float(np.asarray(gamma, dtype=np.float32).reshape(()))

    def rep4(Wm):  # [8, 64] -> [64, 128] with W^T at free-cols 32a..32a+8
        out = np.zeros((C, 128), dtype=_NP_BF16)
        for a in range(4):
            out[:, 32 * a : 32 * a + 8] = Wm.T.astype(_NP_BF16)
        return out

    def rep1(Wm):  # [8, 64] -> [64, 128] with W^T only at free-cols 0..8
        out = np.zeros((C, 128), dtype=_NP_BF16)
        out[:, 0:8] = Wm.T.astype(_NP_BF16)
        return out

    wq4 = rep4(Wq)
    wk4 = rep1(Wk)
    wv = np.ascontiguousarray((gamma * Wv.T).astype(_NP_BF16))  # [64, 64]

    in_maps = []
    for core in range(N_CORES):
        b, h = divmod(core, 2)
        xf = x[b].reshape(C, N)
        xk = xf[:, h * MHALF : (h + 1) * MHALF]
        in_maps.append(
            {
                "xfp": xf.astype(_NP_BF16),
                "xkp": np.ascontiguousarray(xk.astype(_NP_BF16)),
                "xkf": np.ascontiguousarray(xk),
                "wq4": wq4,
                "wk4": wk4,
                "wv": wv,
            }
        )
    return in_maps


def gather(results):
    out = np.empty((B, C, N), dtype=np.float32)
    for core in range(N_CORES):
        b, h = divmod(core, 2)
        out[b][:, h * MHALF : (h + 1) * MHALF] = results[core]["out"]
    return out.reshape(B, C, H, W)


def run(inputs, **spmd_kwargs):
    nc = get_program()
    in_maps = make_in_maps(
        inputs["x"], inputs["Wq"], inputs["Wk"], inputs["Wv"], inputs["gamma"]
    )
    res = run_bass_kernel_spmd(nc, in_maps, core_ids=list(range(N_CORES)), **spmd_kwargs)
    return gather(res.results), res


def kernel(x, Wq, Wk, Wv, gamma):
    out, _ = run({"x": x, "Wq": Wq, "Wk": Wk, "Wv": Wv, "gamma": gamma})
    return out
